# revision 3
# baseline (speedup 1.0000x reference)
"""KANFIS forward on 8 NeuronCores, data-parallel over the batch — v2.

Key differences vs v1:
  * RBF gaussians via exp of a LINEAR form: e = exp(c_k/s^2 * x - 0.5/s^2 * x^2
    + bias), with [x; x^2] stacked on 128 partitions and the quadratic built by
    a single PE matmul (f32r) per k-pair. No per-k ACT Square pass.
  * k-values pair-packed: 4 ACT exps of [128,512] per chunk instead of 8+8
    ops of [64,512].
  * fp16 / f32r matmuls: 1 cycle/row on PE instead of 4 (fp32).
  * Single AllReduce: BN1 stats (S1,Q1 via bn_stats/bn_aggr) plus the
    second-moment matrix M = proj^T proj ride one [128,130] collective;
    BN2 statistics are derived from (S1, M) on-device because layer 2 is
    linear in proj.
  * BN affines folded into matmul weights / activation scale+bias; proj_b and
    fp_b dropped entirely (they cancel inside BatchNorm).
  * z kept feature-major [20-per-chunk rows], gelu packed 4 chunks per ACT op;
    fuzzy memberships and head reduction per 1024-batch pair.
  * Element-wise work spread across ACT / DVE / GPSIMD.
"""
import numpy as np
from contextlib import ExitStack

import concourse.bass as bass
import concourse.tile as tile
from concourse import mybir
from concourse.vector_clock import ScopedClock
from concourse.bass_utils import run_bass_kernel_spmd

F32 = mybir.dt.float32
F32R = mybir.dt.float32r
F16 = mybir.dt.float16
AF = mybir.ActivationFunctionType
ALU = mybir.AluOpType

NCORES = 8
B = 131072
BS = B // NCORES          # 16384 rows per core
G, GS, K, O = 8, 8, 8, 16
TOT, R, FIN = 128, 10, 20
EPS = 1e-5
FC = 512                  # chunk free size
NCH = BS // FC            # 32 chunks
NPAIR = NCH // 2          # 16 chunk-pairs in phase 3b
NG = NCH // 2             # 16 gelu groups of 2 chunks


class SplitDrainTileContext(tile.TileContext):
    """walrus on this stack rejects >1 sync wait per instruction; split the
    kernel-tail drain's waits into single-wait nops."""

    def _drain_and_barrier(self, tick_clock, wait_clock):
        nc = self.nc
        nop = nc.sync.nop(nofuse=True)
        wait_clock.add_sem_waits(nop.ins, ScopedClock({None: tick_clock.global_clock}))
        si = nop.ins.sync_info
        waits = list(si.on_wait) if si and si.on_wait else []
        if len(waits) > 1:
            nop.ins.sync_info = mybir.SyncInfo(on_wait=waits[:1], on_update=si.on_update)
            for w in waits[1:]:
                n2 = nc.sync.nop(nofuse=True)
                n2.ins.sync_info = mybir.SyncInfo(on_wait=[w], on_update=[])
        nc.sync.drain()
        nc.all_engine_barrier()
        assert self.sems is not None
        popped = nc._tile_sem_poison_stack.pop()
        assert popped is self._sem_poison
        nc.clear_and_free_semaphores(list(self.sems.allocated().values()))
        nc.all_engine_barrier()


def _build(p):
    nc = bass.Bass(num_devices=NCORES)
    x = nc.dram_tensor("x", [BS, 64], F32, kind="ExternalInput")
    out = nc.dram_tensor("out", [BS, 1], F32, kind="ExternalOutput")
    ar_in = nc.dram_tensor("ar_in", [128, 130], F16)
    ar_out = nc.dram_tensor("ar_out", [1024, 130], F16)

    # ---- baked constants (numpy) ----
    sig = np.exp(p["rbf_log_widths"]) + 1e-6            # [G,K]
    inv2 = (1.0 / sig ** 2).astype(np.float64)
    cen = p["rbf_centres"].astype(np.float64)
    pw = p["proj_W"]                                    # [G,O,GS]
    rw = p["rbf_weights"]                               # [G,K]
    # u-matmul lhsT [128, 4*128] f32 (used as f32r): rows 0-63 x_f, 64-127 x2_f
    # col (pair p, m = kk*64+f): u = (c/s^2) x - (0.5/s^2) x^2
    ku = np.zeros((128, 4 * 128), np.float32)
    keb = np.zeros((128, 4), np.float32)                # exp bias per pair
    for pp_ in range(4):
        for kk in range(2):
            k = 2 * pp_ + kk
            for f in range(64):
                g = f // GS
                m = kk * 64 + f
                ku[f, pp_ * 128 + m] = cen[g, k] * inv2[g, k]
                ku[64 + f, pp_ * 128 + m] = -0.5 * inv2[g, k]
                keb[m, pp_] = -0.5 * cen[g, k] ** 2 * inv2[g, k]
    # proj lhsT per pair [128, 4*128] f16: rows m=(kk,f) -> cols go.
    # The exp bias exp(-0.5 c^2/s^2) is folded multiplicatively into the rows
    # so all four per-pair exps share scale=1, bias=0 and merge into one op.
    klh = np.zeros((128, 4 * 128), np.float32)
    for pp_ in range(4):
        for kk in range(2):
            k = 2 * pp_ + kk
            for g in range(G):
                eb = np.exp(-0.5 * cen[g, k] ** 2 * inv2[g, k])
                klh[kk * 64 + g * GS:(kk * 64 + (g + 1) * GS),
                    pp_ * 128 + g * O:pp_ * 128 + (g + 1) * O] = \
                    pw[g].T * rw[g, k] * eb
    # linear term lhsT [128,128] (rhs = X2; x2 rows get zero coef)
    klin = np.zeros((128, 128), np.float32)
    for g in range(G):
        klin[g * GS:(g + 1) * GS, g * O:(g + 1) * O] = pw[g].T * p["rbf_linear_w"][g]

    su = np.exp(p["fz_log_su"]) + 1e-6                  # [R,FIN]
    sl = np.minimum(np.exp(p["fz_log_sl"]) + 1e-6, su * 0.9)
    cz = p["fz_centres"]
    # fuzzy u lhsT: z-part and z2-part, [20, 200] each -> split 128/72, tiled x4
    afz_z = np.zeros((FIN, 200), np.float32)
    afz_z2 = np.zeros((FIN, 200), np.float32)
    for r in range(R):
        for f in range(FIN):
            m = r * FIN + f
            afz_z[f, m] = -2.0 * cz[r, f] / su[r, f] ** 2
            afz_z2[f, m] = 1.0 / su[r, f] ** 2
    ubias = (-0.5 * cz ** 2 / su ** 2).reshape(200, 1).astype(np.float32)
    lbias = (-0.5 * cz ** 2 / sl ** 2).reshape(200, 1).astype(np.float32)
    lscale = (-0.5 * (su / sl) ** 2).reshape(200, 1).astype(np.float32)
    wh = np.repeat(p["head_W"].reshape(R, 1) * 0.5 / FIN, FIN, 0).astype(np.float32)
    head_b = float(np.asarray(p["head_b"]).reshape(-1)[0])

    def it(name, arr, dt=np.float32):
        return nc.inline_tensor(np.ascontiguousarray(arr, dt), name=name)

    def pad128(a):
        o = np.zeros((128, a.shape[1]), a.dtype)
        o[:a.shape[0]] = a
        return o

    f32_parts = [
        ("keb", keb), ("id32", np.eye(128, dtype=np.float32)),
        ("ones", np.ones((128, 1), np.float32)),
        ("g1", p["bn1_gamma"].reshape(128, 1)),
        ("b1", p["bn1_beta"].reshape(128, 1)),
        ("fpw", p["fp_W"].T),
        ("ub1", ubias[:128]), ("lb1", lbias[:128]), ("ls1", lscale[:128]),
        ("ub2", pad128(ubias[128:])), ("lb2", pad128(lbias[128:])),
        ("ls2", pad128(lscale[128:])),
        ("g2r", pad128(p["bn2_gamma"].reshape(1, 20))),
        ("b2r", pad128(p["bn2_beta"].reshape(1, 20))),
    ]
    def blk2s(az, az2):
        # rows 0-19: z coefs, 32-51: z^2 coefs; replicated at +64
        o = np.zeros((128, az.shape[1]), np.float32)
        for r_ in range(2):
            o[64 * r_:64 * r_ + FIN] = az
            o[64 * r_ + 32:64 * r_ + 32 + FIN] = az2
        return o

    f16_parts = [
        ("ku", ku.astype(np.float16)),
        ("klh", klh.astype(np.float16)),
        ("klin", klin.astype(np.float16)),
        ("idh", np.eye(128, dtype=np.float16)),
        ("azs1", blk2s(afz_z[:, :128], afz_z2[:, :128]).astype(np.float16)),
        ("azs2", blk2s(afz_z[:, 128:], afz_z2[:, 128:]).astype(np.float16)),
        ("wh1", wh[:128].astype(np.float16)),
        ("wh2", pad128(wh[128:]).astype(np.float16)),
    ]
    f32_off, f16_off = {}, {}
    o = 0
    for nm, a in f32_parts:
        f32_off[nm] = o
        o += a.shape[1]
    nf32 = o
    o = 0
    for nm, a in f16_parts:
        f16_off[nm] = o
        o += a.shape[1]
    nf16 = o
    c_f32 = it("c_f32", np.concatenate([a for _, a in f32_parts], axis=1))
    c_f16 = it("c_f16", np.concatenate([a for _, a in f16_parts], axis=1),
               np.float16)

    octx = ExitStack()

    def sb(n, s, dt=F32):
        return octx.enter_context(nc.sbuf_tensor(n, s, dt))

    projT = sb("projT", [128, BS], F16)        # 4MB persistent
    z_all = sb("z_all", [128, NG * FC], F16)   # z rows 0-19/64-83, z^2 rows 32-51/96-115
    stats = sb("stats", [128, NCH * 6])
    # const sbuf blocks
    kf32 = sb("kf32", [128, nf32])
    kf16 = sb("kf16", [128, nf16], F16)

    def s32(nm, w, rows=128):
        off = f32_off[nm]
        return kf32[0:rows, off:off + w]

    def s16(nm, w, rows=128):
        off = f16_off[nm]
        return kf16[0:rows, off:off + w]

    k_ku = s16("ku", 512); k_keb = s32("keb", 4)
    k_klh = s16("klh", 512); k_klin = s16("klin", 128)
    k_id32 = s32("id32", 128); k_idh = s16("idh", 128)
    k_on = s32("ones", 1)
    k_g1 = s32("g1", 1); k_b1 = s32("b1", 1)
    k_g2r = s32("g2r", 20, 1); k_b2r = s32("b2r", 20, 1)
    k_fpw = s32("fpw", 20)
    k_azs1 = s16("azs1", 128); k_azs2 = s16("azs2", 72)
    k_ub1 = s32("ub1", 1); k_ub2 = s32("ub2", 1, 72)
    k_lb1 = s32("lb1", 1); k_lb2 = s32("lb2", 1, 72)
    k_ls1 = s32("ls1", 1); k_ls2 = s32("ls2", 1, 72)
    k_wh1 = s16("wh1", 1); k_wh2 = s16("wh2", 1, 72)

    def s16_ku(pr):
        off = f16_off["ku"]
        return kf16[:, off + pr * 128:off + (pr + 1) * 128]

    def s16_klh(pr):
        off = f16_off["klh"]
        return kf16[:, off + pr * 128:off + (pr + 1) * 128]

    def s32_keb(pr):
        off = f32_off["keb"]
        return kf32[:, off + pr:off + pr + 1]
    k_hb = sb("k_hb", [128, 1])
    k_e1 = sb("k_e1", [128, 1]); k_er = sb("k_er", [1, 1])
    s1q1 = sb("s1q1", [128, 2], F16)
    mvsb = sb("mvsb", [128, 2])
    msb = sb("msb", [128, 128], F16)
    arsb = sb("arsb", [128, 130])
    agsb = sb("agsb", [128, 1040], F16); scr520 = sb("scr520", [128, 520])
    W3f = sb("W3f", [128, 20]); W3h = sb("W3h", [128, 20], F16)
    a1v = sb("a1v", [128, 1]); d1v = sb("d1v", [128, 1])
    ab2 = sb("ab2", [2, 20]); absb = sb("absb", [20, 2]); ab128 = sb("ab128", [128, 2])
    WTsb = sb("WTsb", [128, 20])
    sm1 = sb("sm1", [128, 1]); sm2 = sb("sm2", [128, 1]); sm3 = sb("sm3", [128, 1])
    r20a = sb("r20a", [1, 20]); r20b = sb("r20b", [1, 20]); r20c = sb("r20c", [1, 20])
    r20d = sb("r20d", [1, 20]); r20e = sb("r20e", [1, 20]); r20f = sb("r20f", [1, 20])

    # ================= phase 1 =================
    with ExitStack() as ctx:
        tc = ctx.enter_context(SplitDrainTileContext(nc))
        nc.sync.dma_start(out=kf32[:], in_=c_f32[:, :])
        nc.sync.dma_start(out=kf16[:], in_=c_f16[:, :])
        nc.vector.memset(k_hb[:], head_b)
        nc.vector.memset(k_e1[:], EPS)
        nc.vector.memset(k_er[:], EPS)
        pool = ctx.enter_context(tc.tile_pool(name="p1", bufs=3))
        ps_x = ctx.enter_context(tc.tile_pool(name="psx", bufs=1, space="PSUM"))
        ps_u = ctx.enter_context(tc.tile_pool(name="psu", bufs=1, space="PSUM"))
        ps_p = ctx.enter_context(tc.tile_pool(name="psp", bufs=1, space="PSUM"))
        ps_b = ctx.enter_context(tc.tile_pool(name="psb", bufs=1, space="PSUM"))
        ps_m = ctx.enter_context(tc.tile_pool(name="psm", bufs=1, space="PSUM"))
        Mps = ps_m.tile([128, 128], F32, tag="M")
        xv = x.rearrange("(c j p) f -> c p j f", j=4, p=128)

        def emit_mwork(c2):
            # covers chunks 2*c2, 2*c2+1 (1024 batch rows)
            ccs = 2 * c2 * FC
            pbp = ps_b.tile([128, 2 * FC], F16, tag="pbp")
            for j in range(8):
                nc.tensor.transpose(pbp[:, j * 128:(j + 1) * 128],
                                    projT[:, ccs + j * 128:ccs + (j + 1) * 128],
                                    k_idh)
            pbs = pool.tile([128, 2 * FC], F16, tag="pbs")
            nc.scalar.activation(pbs[:], pbp[:], AF.Identity, bias=0.0, scale=1.0)
            for j in range(8):
                nc.tensor.matmul(Mps[:], pbs[:, j * 128:(j + 1) * 128],
                                 pbs[:, j * 128:(j + 1) * 128],
                                 start=(c2 == 0 and j == 0),
                                 stop=(c2 == NCH // 2 - 1 and j == 7))

        etiles = {}

        def emit_head(c):
            # x load, transpose, X2 build, u-matmuls, exps for chunk c
            xt4 = pool.tile([128, 256], F32, tag="xt4")
            nc.gpsimd.dma_start(out=xt4[:].rearrange("p (j f) -> p j f", j=4),
                                in_=xv[c])
            xt4h = pool.tile([128, 256], F16, tag="xt4h")
            nc.vector.tensor_copy(xt4h[:], xt4[:])
            xtp = ps_x.tile([64, FC], F16, tag="xtp")
            for j in range(4):
                nc.tensor.transpose(xtp[:, j * 128:(j + 1) * 128],
                                    xt4h[:, j * 64:(j + 1) * 64], k_idh)
            X2 = pool.tile([128, FC], F16, tag="X2")
            nc.vector.tensor_copy(X2[0:64, :], xtp[:])
            nc.gpsimd.tensor_mul(X2[64:128, :], X2[0:64, :], X2[0:64, :])
            u4a = ps_u.tile([128, 2 * FC], F32, tag="u4a")
            u4b = ps_u.tile([128, 2 * FC], F32, tag="u4b")
            for pr in range(4):
                ut = u4a if pr < 2 else u4b
                nc.tensor.matmul(ut[:, (pr % 2) * FC:(pr % 2 + 1) * FC],
                                 s16_ku(pr), X2[:], start=True, stop=True)
            e4a = pool.tile([128, 2 * FC], F16, tag="e4a")
            nc.scalar.activation(e4a[:], u4a[:], AF.Exp, bias=0.0, scale=1.0)
            e4b = pool.tile([128, 2 * FC], F16, tag="e4b")
            nc.scalar.activation(e4b[:], u4b[:], AF.Exp, bias=0.0, scale=1.0)
            etiles[c] = (X2, e4a, e4b)

        def emit_pp(c):
            cs = c * FC
            X2, e4a, e4b = etiles.pop(c)
            pp = ps_p.tile([128, FC], F32, tag="pp")
            nc.tensor.matmul(pp[:], k_klin, X2[:], start=True, stop=False)
            for pr in range(4):
                et = e4a if pr < 2 else e4b
                nc.tensor.matmul(pp[:], s16_klh(pr),
                                 et[:, (pr % 2) * FC:(pr % 2 + 1) * FC],
                                 start=False, stop=(pr == 3))
            nc.vector.tensor_copy(projT[:, cs:cs + FC], pp[:])
            nc.vector.bn_stats(stats[:, c * 6:(c + 1) * 6], pp[:])

        for c in range(NCH + 1):
            if c < NCH:
                emit_head(c)
            if c > 0:
                emit_pp(c - 1)
            if c > 2 and c % 2 == 1:
                emit_mwork(c // 2 - 1)
        emit_mwork(NCH // 2 - 1)
        # local BN1 stats -> S1, Q1
        nc.vector.bn_aggr(mvsb[:], stats[:].rearrange("p (r s) -> p r s", s=6))
        nc.vector.tensor_mul(sm1[:], mvsb[:, 0:1], mvsb[:, 0:1])      # mean^2
        nc.vector.tensor_add(sm2[:], mvsb[:, 1:2], sm1[:])            # var+mean^2
        nc.vector.tensor_scalar_mul(s1q1[:, 0:1], mvsb[:, 0:1], float(BS) / 64.0)
        nc.vector.tensor_scalar_mul(s1q1[:, 1:2], sm2[:], float(BS) / 64.0)
        nc.vector.tensor_scalar_mul(msb[:], Mps[:], 1.0 / 64.0)
        nc.sync.dma_start(out=ar_in[:, 0:2], in_=s1q1[:])
        nc.sync.dma_start(out=ar_in[:, 2:130], in_=msb[:])

    with nc.semaphore("cc1") as cs_:
        nc.gpsimd.collective_compute(
            "AllGather", ALU.bypass, replica_groups=[list(range(NCORES))],
            ins=[ar_in[:, :].opt()], outs=[ar_out[:, :].opt()]).then_inc(cs_, 1)
        nc.gpsimd.wait_ge(cs_, 1)
        nc.all_engine_barrier()

    # ================= phase 2 (tiny, derive BN1+BN2 affines) ============
    with ExitStack() as ctx:
        tc = ctx.enter_context(SplitDrainTileContext(nc))
        pool = ctx.enter_context(tc.tile_pool(name="p2", bufs=1))
        psum = ctx.enter_context(tc.tile_pool(name="ps2", bufs=1, space="PSUM"))
        nc.sync.dma_start(out=agsb[:].rearrange("p (i c) -> p i c", i=8),
                          in_=ar_out.rearrange("(i p) c -> p i c", i=8))
        nc.vector.tensor_add(scr520[:], agsb[:, 0:520], agsb[:, 520:1040])
        nc.vector.tensor_add(scr520[:, 0:260], scr520[:, 0:260], scr520[:, 260:520])
        nc.vector.tensor_add(arsb[:], scr520[:, 0:130], scr520[:, 130:260])
        nc.vector.tensor_scalar_mul(arsb[:], arsb[:], 64.0)
        mu = pool.tile([128, 1], F32, tag="mu")
        nc.vector.tensor_scalar_mul(mu[:], arsb[:, 0:1], 1.0 / B)
        nc.vector.tensor_mul(sm1[:], mu[:], mu[:])
        var1 = pool.tile([128, 1], F32, tag="var1")
        nc.vector.scalar_tensor_tensor(var1[:], arsb[:, 1:2], 1.0 / B, sm1[:],
                                       ALU.mult, ALU.subtract)
        lv = pool.tile([128, 1], F32, tag="lv")
        nc.scalar.activation(lv[:], var1[:], AF.Ln, bias=k_e1[:], scale=1.0)
        rs1 = pool.tile([128, 1], F32, tag="rs1")
        nc.scalar.activation(rs1[:], lv[:], AF.Exp, bias=0.0, scale=-0.5)
        nc.vector.tensor_mul(a1v[:], rs1[:], k_g1)
        nc.vector.tensor_mul(sm3[:], mu[:], a1v[:])
        nc.vector.scalar_tensor_tensor(d1v[:], sm3[:], -1.0, k_b1,
                                       ALU.mult, ALU.add)
        nc.scalar.activation(W3f[:], k_fpw, AF.Identity, bias=0.0, scale=a1v[:])
        nc.vector.tensor_copy(W3h[:], W3f[:])
        T1 = psum.tile([128, 20], F32, tag="T1")
        nc.tensor.matmul(T1[:], arsb[:, 2:130], W3f[:], start=True, stop=True)
        nc.vector.tensor_mul(WTsb[:], W3f[:], T1[:])
        qw = psum.tile([1, 20], F32, tag="qw")
        nc.tensor.matmul(qw[:], k_on, WTsb[:], start=True, stop=True)
        cy = psum.tile([1, 20], F32, tag="cy")
        nc.tensor.matmul(cy[:], d1v[:], k_fpw, start=True, stop=True)
        sw = psum.tile([1, 20], F32, tag="sw")
        nc.tensor.matmul(sw[:], arsb[:, 0:1], W3f[:], start=True, stop=True)
        nc.vector.tensor_copy(r20a[:], cy[:])       # cy
        nc.vector.tensor_copy(r20b[:], sw[:])       # S1@W3
        nc.vector.tensor_copy(r20c[:], qw[:])       # diag(W3^T M W3)
        # mu2 = sw/B + cy
        mu2 = r20d
        nc.vector.scalar_tensor_tensor(mu2[:], r20b[:], 1.0 / B, r20a[:],
                                       ALU.mult, ALU.add)
        # q2b = qw/B + (2/B)*cy*sw + cy^2
        t1 = r20e
        nc.vector.tensor_mul(t1[:], r20a[:], r20b[:])        # cy*sw
        t2 = r20f
        nc.vector.tensor_mul(t2[:], r20a[:], r20a[:])        # cy^2
        nc.vector.scalar_tensor_tensor(t1[:], t1[:], 2.0 / B, t2[:],
                                       ALU.mult, ALU.add)
        nc.vector.scalar_tensor_tensor(t1[:], r20c[:], 1.0 / B, t1[:],
                                       ALU.mult, ALU.add)    # q2b
        nc.vector.tensor_mul(t2[:], mu2[:], mu2[:])
        var2 = r20c
        nc.vector.tensor_sub(var2[:], t1[:], t2[:])
        lv2 = r20e
        nc.scalar.activation(lv2[:], var2[:], AF.Ln, bias=k_er[:], scale=1.0)
        rs2 = r20f
        nc.scalar.activation(rs2[:], lv2[:], AF.Exp, bias=0.0, scale=-0.5)
        a2r = r20b
        nc.vector.tensor_mul(a2r[:], rs2[:], k_g2r)                  # a2 row
        t3 = r20e
        nc.vector.tensor_sub(t3[:], r20a[:], mu2[:])                    # cy-mu2
        nc.vector.tensor_mul(t3[:], t3[:], a2r[:])
        b2r_ = r20f
        nc.vector.tensor_add(b2r_[:], t3[:], k_b2r)                  # bias row
        abT = psum.tile([20, 2], F32, tag="abT")
        nc.tensor.transpose(abT[:, 0:1], a2r[:], k_id32[0:1, 0:1])
        nc.tensor.transpose(abT[:, 1:2], b2r_[:], k_id32[0:1, 0:1])
        nc.vector.tensor_copy(absb[:], abT[:])
        nc.vector.memset(ab128[:], 0.0)
        for rr in range(2):
            nc.sync.dma_start(out=ab128[64 * rr:64 * rr + 20, :], in_=absb[:])

    # ================= phase 3 =================
    with ExitStack() as ctx:
        tc = ctx.enter_context(SplitDrainTileContext(nc))
        # --- 3a: z = gelu(a2*zpre+bias), z^2; 4 chunks per ACT op ---
        with ExitStack() as ctx_a:
            ps_z = ctx_a.enter_context(tc.tile_pool(name="psz", bufs=2, space="PSUM"))
            for q in range(NCH // 4):
                zp4 = ps_z.tile([128, 2 * FC], F32, tag="zp4")
                for r in range(4):
                    c = 4 * q + r
                    nc.tensor.matmul(zp4[64 * (c % 2):64 * (c % 2) + 20,
                                         (r // 2) * FC:(r // 2 + 1) * FC],
                                     W3h[:], projT[:, c * FC:(c + 1) * FC],
                                     start=True, stop=True)
                gs = q * 2 * FC
                nc.scalar.activation(z_all[:, gs:gs + 2 * FC], zp4[:], AF.Gelu,
                                     bias=ab128[:, 1:2], scale=ab128[:, 0:1])
                for r in range(2):
                    zr = 64 * r
                    nc.vector.tensor_mul(z_all[zr + 32:zr + 52, gs:gs + 2 * FC],
                                         z_all[zr:zr + 20, gs:gs + 2 * FC],
                                         z_all[zr:zr + 20, gs:gs + 2 * FC])
        # --- 3b: fuzzy memberships + head, per chunk-pair (1024 batch) ---
        pool = ctx.enter_context(tc.tile_pool(name="p3", bufs=3))
        ps_u1 = ctx.enter_context(tc.tile_pool(name="psu1", bufs=2, space="PSUM"))
        ps_u2 = ctx.enter_context(tc.tile_pool(name="psu2", bufs=1, space="PSUM"))
        ps_o = ctx.enter_context(tc.tile_pool(name="pso", bufs=1, space="PSUM"))
        orow = {}
        utiles = {}

        def emit_umm(i):
            u1 = ps_u1.tile([128, 2 * FC], F32, tag="u1")
            u2 = ps_u2.tile([72, 2 * FC], F32, tag="u2")
            utiles[i] = (u1, u2)
            for h in range(2):
                c = 2 * i + h
                g, r = c // 2, c % 2
                zsl = z_all[64 * r:64 * r + 52, g * FC:(g + 1) * FC]
                hs = h * FC
                nc.tensor.matmul(u1[:, hs:hs + FC],
                                 k_azs1[64 * r:64 * r + 52, :],
                                 zsl, start=True, stop=True)
                nc.tensor.matmul(u2[:, hs:hs + FC],
                                 k_azs2[64 * r:64 * r + 52, :],
                                 zsl, start=True, stop=True)

        def emit_tail(i):
            u1, u2 = utiles.pop(i)
            e1u = pool.tile([128, 2 * FC], F16, tag="e1u")
            nc.scalar.activation(e1u[:], u1[:], AF.Exp, bias=k_ub1, scale=-0.5)
            e1l = pool.tile([128, 2 * FC], F16, tag="e1l")
            nc.scalar.activation(e1l[:], u1[:], AF.Exp, bias=k_lb1, scale=k_ls1)
            e2u = pool.tile([72, 2 * FC], F16, tag="e2u")
            nc.scalar.activation(e2u[:], u2[:], AF.Exp, bias=k_ub2, scale=-0.5)
            e2l = pool.tile([72, 2 * FC], F16, tag="e2l")
            nc.scalar.activation(e2l[:], u2[:], AF.Exp, bias=k_lb2, scale=k_ls2)
            if i % 2 == 0:
                orow_t = ps_o.tile([128, 2 * FC], F32, tag="orow")
                orow[0] = orow_t
            rr = 64 * (i % 2)
            for h in range(2):
                hs = h * FC
                nc.tensor.matmul(orow[0][rr:rr + 1, hs:hs + FC], k_wh1,
                                 e1u[:, hs:hs + FC], start=True, stop=False)
                nc.tensor.matmul(orow[0][rr:rr + 1, hs:hs + FC], k_wh1,
                                 e1l[:, hs:hs + FC], start=False, stop=False)
                nc.tensor.matmul(orow[0][rr:rr + 1, hs:hs + FC], k_wh2,
                                 e2u[:, hs:hs + FC], start=False, stop=False)
                nc.tensor.matmul(orow[0][rr:rr + 1, hs:hs + FC], k_wh2,
                                 e2l[:, hs:hs + FC], start=False, stop=True)
            if i % 2 == 1:
                g2 = i // 2
                outsb = pool.tile([128, 2 * FC], F32, tag="outsb")
                nc.vector.tensor_scalar_add(outsb[:], orow[0][:], k_hb[:])
                nc.sync.dma_start(
                    out=out[:, :].rearrange("(g r q) one -> g r (q one)",
                                            r=2, q=2 * FC)[g2],
                    in_=outsb[:].rearrange("(r k) q -> r k q", k=64)[:, 0])

        for i in range(NPAIR + 1):
            if i < NPAIR:
                emit_umm(i)
            if i > 0:
                emit_tail(i - 1)
    octx.close()
    _split_multiwaits(nc)
    return nc


def _split_multiwaits(nc, max_waits=1):
    # hoist extra sync waits into single-wait nops placed just before the
    # offending instruction (walrus here rejects multi-wait instructions)
    for bb in nc.m.functions[0].blocks:
        insts = bb.instructions
        i = 0
        while i < len(insts):
            inst = insts[i]
            si = getattr(inst, "sync_info", None)
            waits = list(si.on_wait) if si and si.on_wait else []
            if len(waits) > max_waits:
                inst.sync_info = mybir.SyncInfo(
                    on_wait=waits[:max_waits], on_update=si.on_update)
                for j, w in enumerate(waits[max_waits:]):
                    n = mybir.InstNoOp(name=f"{inst.name}_ws{j}", ins=[], outs=[])
                    n.engine = inst.engine
                    n.sync_info = mybir.SyncInfo(on_wait=[w], on_update=[])
                    nc.register_instruction(n, overwrite=True)
                    insts.insert(i, n)
                    i += 1
            i += 1


LAST = None


def kernel(**inputs):
    global LAST
    import os
    x = np.asarray(inputs["x"], np.float32)
    p = {k: np.asarray(v) for k, v in inputs.items() if k != "x"}
    nc = _build(p)
    in_maps = [{"x": np.ascontiguousarray(x[i * BS:(i + 1) * BS])}
               for i in range(NCORES)]
    kw = {}
    tdir = os.environ.get("KANFIS_TRACE")
    if tdir:
        os.makedirs(tdir, exist_ok=True)
        kw = dict(trace=True, tmpdir=tdir)
    res = run_bass_kernel_spmd(nc, in_maps, core_ids=list(range(NCORES)), **kw)
    LAST = res
    return np.concatenate([res.results[i]["out"] for i in range(NCORES)], axis=0)


# revision 4
# speedup vs baseline: 1.0187x; 1.0187x over previous
"""KANFIS forward on 8 NeuronCores, data-parallel over the batch — v2.

Key differences vs v1:
  * RBF gaussians via exp of a LINEAR form: e = exp(c_k/s^2 * x - 0.5/s^2 * x^2
    + bias), with [x; x^2] stacked on 128 partitions and the quadratic built by
    a single PE matmul (f32r) per k-pair. No per-k ACT Square pass.
  * k-values pair-packed: 4 ACT exps of [128,512] per chunk instead of 8+8
    ops of [64,512].
  * fp16 / f32r matmuls: 1 cycle/row on PE instead of 4 (fp32).
  * Single AllReduce: BN1 stats (S1,Q1 via bn_stats/bn_aggr) plus the
    second-moment matrix M = proj^T proj ride one [128,130] collective;
    BN2 statistics are derived from (S1, M) on-device because layer 2 is
    linear in proj.
  * BN affines folded into matmul weights / activation scale+bias; proj_b and
    fp_b dropped entirely (they cancel inside BatchNorm).
  * z kept feature-major [20-per-chunk rows], gelu packed 4 chunks per ACT op;
    fuzzy memberships and head reduction per 1024-batch pair.
  * Element-wise work spread across ACT / DVE / GPSIMD.
"""
import numpy as np
from contextlib import ExitStack

import concourse.bass as bass
import concourse.tile as tile
from concourse import mybir
from concourse.vector_clock import ScopedClock
from concourse.bass_utils import run_bass_kernel_spmd

F32 = mybir.dt.float32
F32R = mybir.dt.float32r
F16 = mybir.dt.float16
AF = mybir.ActivationFunctionType
ALU = mybir.AluOpType

NCORES = 8
B = 131072
BS = B // NCORES          # 16384 rows per core
G, GS, K, O = 8, 8, 8, 16
TOT, R, FIN = 128, 10, 20
EPS = 1e-5
FC = 512                  # chunk free size
NCH = BS // FC            # 32 chunks
NPAIR = NCH // 2          # 16 chunk-pairs in phase 3b
NG = NCH // 2             # 16 gelu groups of 2 chunks


class SplitDrainTileContext(tile.TileContext):
    """walrus on this stack rejects >1 sync wait per instruction; split the
    kernel-tail drain's waits into single-wait nops."""

    def _drain_and_barrier(self, tick_clock, wait_clock):
        nc = self.nc
        nop = nc.sync.nop(nofuse=True)
        wait_clock.add_sem_waits(nop.ins, ScopedClock({None: tick_clock.global_clock}))
        si = nop.ins.sync_info
        waits = list(si.on_wait) if si and si.on_wait else []
        if len(waits) > 1:
            nop.ins.sync_info = mybir.SyncInfo(on_wait=waits[:1], on_update=si.on_update)
            for w in waits[1:]:
                n2 = nc.sync.nop(nofuse=True)
                n2.ins.sync_info = mybir.SyncInfo(on_wait=[w], on_update=[])
        nc.sync.drain()
        nc.all_engine_barrier()
        assert self.sems is not None
        popped = nc._tile_sem_poison_stack.pop()
        assert popped is self._sem_poison
        nc.clear_and_free_semaphores(list(self.sems.allocated().values()))
        nc.all_engine_barrier()


def _build(p):
    nc = bass.Bass(num_devices=NCORES)
    x = nc.dram_tensor("x", [BS, 64], F32, kind="ExternalInput")
    out = nc.dram_tensor("out", [BS, 1], F32, kind="ExternalOutput")
    ar_in = nc.dram_tensor("ar_in", [128, 130], F16)
    ar_out = nc.dram_tensor("ar_out", [1024, 130], F16)

    # ---- baked constants (numpy) ----
    sig = np.exp(p["rbf_log_widths"]) + 1e-6            # [G,K]
    inv2 = (1.0 / sig ** 2).astype(np.float64)
    cen = p["rbf_centres"].astype(np.float64)
    pw = p["proj_W"]                                    # [G,O,GS]
    rw = p["rbf_weights"]                               # [G,K]
    # u-matmul lhsT [128, 4*128] f32 (used as f32r): rows 0-63 x_f, 64-127 x2_f
    # col (pair p, m = kk*64+f): u = (c/s^2) x - (0.5/s^2) x^2
    ku = np.zeros((128, 4 * 128), np.float32)
    keb = np.zeros((128, 4), np.float32)                # exp bias per pair
    for pp_ in range(4):
        for kk in range(2):
            k = 2 * pp_ + kk
            for f in range(64):
                g = f // GS
                m = kk * 64 + f
                ku[f, pp_ * 128 + m] = cen[g, k] * inv2[g, k]
                ku[64 + f, pp_ * 128 + m] = -0.5 * inv2[g, k]
                keb[m, pp_] = -0.5 * cen[g, k] ** 2 * inv2[g, k]
    # proj lhsT per pair [128, 4*128] f16: rows m=(kk,f) -> cols go.
    # The exp bias exp(-0.5 c^2/s^2) is folded multiplicatively into the rows
    # so all four per-pair exps share scale=1, bias=0 and merge into one op.
    klh = np.zeros((128, 4 * 128), np.float32)
    for pp_ in range(4):
        for kk in range(2):
            k = 2 * pp_ + kk
            for g in range(G):
                eb = np.exp(-0.5 * cen[g, k] ** 2 * inv2[g, k])
                klh[kk * 64 + g * GS:(kk * 64 + (g + 1) * GS),
                    pp_ * 128 + g * O:pp_ * 128 + (g + 1) * O] = \
                    pw[g].T * rw[g, k] * eb
    # linear term lhsT [128,128] (rhs = X2; x2 rows get zero coef)
    klin = np.zeros((128, 128), np.float32)
    for g in range(G):
        klin[g * GS:(g + 1) * GS, g * O:(g + 1) * O] = pw[g].T * p["rbf_linear_w"][g]

    su = np.exp(p["fz_log_su"]) + 1e-6                  # [R,FIN]
    sl = np.minimum(np.exp(p["fz_log_sl"]) + 1e-6, su * 0.9)
    cz = p["fz_centres"]
    # fuzzy u lhsT: z-part and z2-part, [20, 200] each -> split 128/72, tiled x4
    afz_z = np.zeros((FIN, 200), np.float32)
    afz_z2 = np.zeros((FIN, 200), np.float32)
    for r in range(R):
        for f in range(FIN):
            m = r * FIN + f
            afz_z[f, m] = -2.0 * cz[r, f] / su[r, f] ** 2
            afz_z2[f, m] = 1.0 / su[r, f] ** 2
    ubias = (-0.5 * cz ** 2 / su ** 2).reshape(200, 1).astype(np.float32)
    lbias = (-0.5 * cz ** 2 / sl ** 2).reshape(200, 1).astype(np.float32)
    lscale = (-0.5 * (su / sl) ** 2).reshape(200, 1).astype(np.float32)
    wh = np.repeat(p["head_W"].reshape(R, 1) * 0.5 / FIN, FIN, 0).astype(np.float32)
    head_b = float(np.asarray(p["head_b"]).reshape(-1)[0])

    def it(name, arr, dt=np.float32):
        return nc.inline_tensor(np.ascontiguousarray(arr, dt), name=name)

    def pad128(a):
        o = np.zeros((128, a.shape[1]), a.dtype)
        o[:a.shape[0]] = a
        return o

    f32_parts = [
        ("keb", keb), ("id32", np.eye(128, dtype=np.float32)),
        ("ones", np.ones((128, 1), np.float32)),
        ("g1", p["bn1_gamma"].reshape(128, 1)),
        ("b1", p["bn1_beta"].reshape(128, 1)),
        ("fpw", p["fp_W"].T),
        ("ub1", ubias[:128]), ("lb1", lbias[:128]), ("ls1", lscale[:128]),
        ("ub2", pad128(ubias[128:])), ("lb2", pad128(lbias[128:])),
        ("ls2", pad128(lscale[128:])),
        ("g2r", pad128(p["bn2_gamma"].reshape(1, 20))),
        ("b2r", pad128(p["bn2_beta"].reshape(1, 20))),
    ]
    def blk2s(az, az2):
        # rows 0-19: z coefs, 32-51: z^2 coefs; replicated at +64
        o = np.zeros((128, az.shape[1]), np.float32)
        for r_ in range(2):
            o[64 * r_:64 * r_ + FIN] = az
            o[64 * r_ + 32:64 * r_ + 32 + FIN] = az2
        return o

    f16_parts = [
        ("ku", ku.astype(np.float16)),
        ("klh", klh.astype(np.float16)),
        ("klin", klin.astype(np.float16)),
        ("idh", np.eye(128, dtype=np.float16)),
        ("azs1", blk2s(afz_z[:, :128], afz_z2[:, :128]).astype(np.float16)),
        ("azs2", blk2s(afz_z[:, 128:], afz_z2[:, 128:]).astype(np.float16)),
        ("wh1", wh[:128].astype(np.float16)),
        ("wh2", pad128(wh[128:]).astype(np.float16)),
    ]
    f32_off, f16_off = {}, {}
    o = 0
    for nm, a in f32_parts:
        f32_off[nm] = o
        o += a.shape[1]
    nf32 = o
    o = 0
    for nm, a in f16_parts:
        f16_off[nm] = o
        o += a.shape[1]
    nf16 = o
    c_f32 = it("c_f32", np.concatenate([a for _, a in f32_parts], axis=1))
    c_f16 = it("c_f16", np.concatenate([a for _, a in f16_parts], axis=1),
               np.float16)

    octx = ExitStack()

    def sb(n, s, dt=F32):
        return octx.enter_context(nc.sbuf_tensor(n, s, dt))

    projT = sb("projT", [128, BS], F16)        # 4MB persistent
    z_all = sb("z_all", [128, NG * FC], F16)   # z rows 0-19/64-83, z^2 rows 32-51/96-115
    stats = sb("stats", [128, NCH * 6])
    # const sbuf blocks
    kf32 = sb("kf32", [128, nf32])
    kf16 = sb("kf16", [128, nf16], F16)

    def s32(nm, w, rows=128):
        off = f32_off[nm]
        return kf32[0:rows, off:off + w]

    def s16(nm, w, rows=128):
        off = f16_off[nm]
        return kf16[0:rows, off:off + w]

    k_ku = s16("ku", 512); k_keb = s32("keb", 4)
    k_klh = s16("klh", 512); k_klin = s16("klin", 128)
    k_id32 = s32("id32", 128); k_idh = s16("idh", 128)
    k_on = s32("ones", 1)
    k_g1 = s32("g1", 1); k_b1 = s32("b1", 1)
    k_g2r = s32("g2r", 20, 1); k_b2r = s32("b2r", 20, 1)
    k_fpw = s32("fpw", 20)
    k_azs1 = s16("azs1", 128); k_azs2 = s16("azs2", 72)
    k_ub1 = s32("ub1", 1); k_ub2 = s32("ub2", 1, 72)
    k_lb1 = s32("lb1", 1); k_lb2 = s32("lb2", 1, 72)
    k_ls1 = s32("ls1", 1); k_ls2 = s32("ls2", 1, 72)
    k_wh1 = s16("wh1", 1); k_wh2 = s16("wh2", 1, 72)

    def s16_ku(pr):
        off = f16_off["ku"]
        return kf16[:, off + pr * 128:off + (pr + 1) * 128]

    def s16_klh(pr):
        off = f16_off["klh"]
        return kf16[:, off + pr * 128:off + (pr + 1) * 128]

    def s32_keb(pr):
        off = f32_off["keb"]
        return kf32[:, off + pr:off + pr + 1]
    k_hb = sb("k_hb", [128, 1])
    k_e1 = sb("k_e1", [128, 1]); k_er = sb("k_er", [1, 1])
    s1q1 = sb("s1q1", [128, 2], F16)
    mvsb = sb("mvsb", [128, 2])
    msb = sb("msb", [128, 128], F16)
    arsb = sb("arsb", [128, 130])
    agsb = sb("agsb", [128, 1040], F16); scr520 = sb("scr520", [128, 520])
    W3f = sb("W3f", [128, 20]); W3h = sb("W3h", [128, 20], F16)
    a1v = sb("a1v", [128, 1]); d1v = sb("d1v", [128, 1])
    ab2 = sb("ab2", [2, 20]); absb = sb("absb", [20, 2]); ab128 = sb("ab128", [128, 2])
    WTsb = sb("WTsb", [128, 20])
    sm1 = sb("sm1", [128, 1]); sm2 = sb("sm2", [128, 1]); sm3 = sb("sm3", [128, 1])
    r20a = sb("r20a", [1, 20]); r20b = sb("r20b", [1, 20]); r20c = sb("r20c", [1, 20])
    r20d = sb("r20d", [1, 20]); r20e = sb("r20e", [1, 20]); r20f = sb("r20f", [1, 20])

    # ================= phase 1 =================
    with ExitStack() as ctx:
        tc = ctx.enter_context(SplitDrainTileContext(nc))
        nc.sync.dma_start(out=kf32[:], in_=c_f32[:, :])
        nc.sync.dma_start(out=kf16[:], in_=c_f16[:, :])
        nc.vector.memset(k_hb[:], head_b)
        nc.vector.memset(k_e1[:], EPS)
        nc.vector.memset(k_er[:], EPS)
        pool = ctx.enter_context(tc.tile_pool(name="p1", bufs=3))
        poolx = ctx.enter_context(tc.tile_pool(name="p1x", bufs=6))
        ps_x = ctx.enter_context(tc.tile_pool(name="psx", bufs=1, space="PSUM"))
        ps_u = ctx.enter_context(tc.tile_pool(name="psu", bufs=1, space="PSUM"))
        ps_p = ctx.enter_context(tc.tile_pool(name="psp", bufs=1, space="PSUM"))
        ps_b = ctx.enter_context(tc.tile_pool(name="psb", bufs=1, space="PSUM"))
        ps_m = ctx.enter_context(tc.tile_pool(name="psm", bufs=1, space="PSUM"))
        Mps = ps_m.tile([128, 128], F32, tag="M")
        xv = x.rearrange("(c j p) f -> c p j f", j=4, p=128)

        def emit_mwork(c2):
            # covers chunks 2*c2, 2*c2+1 (1024 batch rows)
            ccs = 2 * c2 * FC
            pbp = ps_b.tile([128, 2 * FC], F16, tag="pbp")
            for j in range(8):
                nc.tensor.transpose(pbp[:, j * 128:(j + 1) * 128],
                                    projT[:, ccs + j * 128:ccs + (j + 1) * 128],
                                    k_idh)
            pbs = pool.tile([128, 2 * FC], F16, tag="pbs")
            nc.scalar.activation(pbs[:], pbp[:], AF.Identity, bias=0.0, scale=1.0)
            for j in range(8):
                nc.tensor.matmul(Mps[:], pbs[:, j * 128:(j + 1) * 128],
                                 pbs[:, j * 128:(j + 1) * 128],
                                 start=(c2 == 0 and j == 0),
                                 stop=(c2 == NCH // 2 - 1 and j == 7))

        etiles = {}

        def emit_head(c):
            # x load, transpose, X2 build, u-matmuls, exps for chunk c
            xt4 = poolx.tile([128, 256], F32, tag="xt4")
            nc.gpsimd.dma_start(out=xt4[:].rearrange("p (j f) -> p j f", j=4),
                                in_=xv[c])
            xt4h = pool.tile([128, 256], F16, tag="xt4h")
            nc.vector.tensor_copy(xt4h[:], xt4[:])
            xtp = ps_x.tile([64, FC], F16, tag="xtp")
            for j in range(4):
                nc.tensor.transpose(xtp[:, j * 128:(j + 1) * 128],
                                    xt4h[:, j * 64:(j + 1) * 64], k_idh)
            X2 = pool.tile([128, FC], F16, tag="X2")
            nc.vector.tensor_copy(X2[0:64, :], xtp[:])
            nc.gpsimd.tensor_mul(X2[64:128, :], X2[0:64, :], X2[0:64, :])
            u4a = ps_u.tile([128, 2 * FC], F32, tag="u4a")
            u4b = ps_u.tile([128, 2 * FC], F32, tag="u4b")
            for pr in range(4):
                ut = u4a if pr < 2 else u4b
                nc.tensor.matmul(ut[:, (pr % 2) * FC:(pr % 2 + 1) * FC],
                                 s16_ku(pr), X2[:], start=True, stop=True)
            e4a = pool.tile([128, 2 * FC], F16, tag="e4a")
            nc.scalar.activation(e4a[:], u4a[:], AF.Exp, bias=0.0, scale=1.0)
            e4b = pool.tile([128, 2 * FC], F16, tag="e4b")
            nc.scalar.activation(e4b[:], u4b[:], AF.Exp, bias=0.0, scale=1.0)
            etiles[c] = (X2, e4a, e4b)

        def emit_pp(c):
            cs = c * FC
            X2, e4a, e4b = etiles.pop(c)
            pp = ps_p.tile([128, FC], F32, tag="pp")
            nc.tensor.matmul(pp[:], k_klin, X2[:], start=True, stop=False)
            for pr in range(4):
                et = e4a if pr < 2 else e4b
                nc.tensor.matmul(pp[:], s16_klh(pr),
                                 et[:, (pr % 2) * FC:(pr % 2 + 1) * FC],
                                 start=False, stop=(pr == 3))
            nc.vector.tensor_copy(projT[:, cs:cs + FC], pp[:])
            nc.vector.bn_stats(stats[:, c * 6:(c + 1) * 6], pp[:])

        for c in range(NCH + 1):
            if c < NCH:
                emit_head(c)
            if c > 0:
                emit_pp(c - 1)
            if c > 2 and c % 2 == 1:
                emit_mwork(c // 2 - 1)
        emit_mwork(NCH // 2 - 1)
        # local BN1 stats -> S1, Q1
        nc.vector.bn_aggr(mvsb[:], stats[:].rearrange("p (r s) -> p r s", s=6))
        nc.vector.tensor_mul(sm1[:], mvsb[:, 0:1], mvsb[:, 0:1])      # mean^2
        nc.vector.tensor_add(sm2[:], mvsb[:, 1:2], sm1[:])            # var+mean^2
        nc.vector.tensor_scalar_mul(s1q1[:, 0:1], mvsb[:, 0:1], float(BS) / 64.0)
        nc.vector.tensor_scalar_mul(s1q1[:, 1:2], sm2[:], float(BS) / 64.0)
        nc.vector.tensor_scalar_mul(msb[:], Mps[:], 1.0 / 64.0)
        nc.sync.dma_start(out=ar_in[:, 0:2], in_=s1q1[:])
        nc.sync.dma_start(out=ar_in[:, 2:130], in_=msb[:])

    with nc.semaphore("cc1") as cs_:
        nc.gpsimd.collective_compute(
            "AllGather", ALU.bypass, replica_groups=[list(range(NCORES))],
            ins=[ar_in[:, :].opt()], outs=[ar_out[:, :].opt()]).then_inc(cs_, 1)
        nc.gpsimd.wait_ge(cs_, 1)
        nc.all_engine_barrier()

    # ================= phase 3 =================
    with ExitStack() as ctx:
        tc = ctx.enter_context(SplitDrainTileContext(nc))
        # --- phase 2: derive BN1 scale/shift and BN2 stats from (S1,Q1,M) ---
        with ExitStack() as ctx2:
            pool2 = ctx2.enter_context(tc.tile_pool(name="p2", bufs=1))
            ps2 = ctx2.enter_context(tc.tile_pool(name="ps2", bufs=1, space="PSUM"))
            nc.sync.dma_start(out=agsb[:].rearrange("p (i c) -> p i c", i=8),
                              in_=ar_out.rearrange("(i p) c -> p i c", i=8))
            nc.vector.tensor_add(scr520[:], agsb[:, 0:520], agsb[:, 520:1040])
            nc.vector.tensor_add(scr520[:, 0:260], scr520[:, 0:260],
                                 scr520[:, 260:520])
            nc.vector.tensor_add(arsb[:], scr520[:, 0:130], scr520[:, 130:260])
            nc.vector.tensor_scalar_mul(arsb[:], arsb[:], 64.0)
            mu = pool2.tile([128, 1], F32, tag="mu")
            nc.vector.tensor_scalar_mul(mu[:], arsb[:, 0:1], 1.0 / B)
            nc.vector.tensor_mul(sm1[:], mu[:], mu[:])
            var1 = pool2.tile([128, 1], F32, tag="var1")
            nc.vector.scalar_tensor_tensor(var1[:], arsb[:, 1:2], 1.0 / B, sm1[:],
                                           ALU.mult, ALU.subtract)
            lv = pool2.tile([128, 1], F32, tag="lv")
            nc.scalar.activation(lv[:], var1[:], AF.Ln, bias=k_e1[:], scale=1.0)
            rs1 = pool2.tile([128, 1], F32, tag="rs1")
            nc.scalar.activation(rs1[:], lv[:], AF.Exp, bias=0.0, scale=-0.5)
            nc.vector.tensor_mul(a1v[:], rs1[:], k_g1)
            nc.vector.tensor_mul(sm3[:], mu[:], a1v[:])
            nc.vector.scalar_tensor_tensor(d1v[:], sm3[:], -1.0, k_b1,
                                           ALU.mult, ALU.add)
            nc.scalar.activation(W3f[:], k_fpw, AF.Identity, bias=0.0, scale=a1v[:])
            nc.vector.tensor_copy(W3h[:], W3f[:])
            p2ps = ps2.tile([128, 128], F32, tag="p2ps")
            T1 = p2ps[:, 0:20]
            qw = p2ps[0:1, 20:40]
            cy = p2ps[0:1, 40:60]
            sw = p2ps[0:1, 60:80]
            abT = p2ps[0:20, 80:82]
            nc.tensor.matmul(T1, arsb[:, 2:130], W3f[:], start=True, stop=True)
            nc.vector.tensor_mul(WTsb[:], W3f[:], T1)
            nc.tensor.matmul(qw, k_on, WTsb[:], start=True, stop=True)
            nc.tensor.matmul(cy, d1v[:], k_fpw, start=True, stop=True)
            nc.tensor.matmul(sw, arsb[:, 0:1], W3f[:], start=True, stop=True)
            nc.vector.tensor_copy(r20a[:], cy)       # cy
            nc.vector.tensor_copy(r20b[:], sw)       # S1@W3
            nc.vector.tensor_copy(r20c[:], qw)       # diag(W3^T M W3)
            mu2 = r20d
            nc.vector.scalar_tensor_tensor(mu2[:], r20b[:], 1.0 / B, r20a[:],
                                           ALU.mult, ALU.add)
            t1 = r20e
            nc.vector.tensor_mul(t1[:], r20a[:], r20b[:])
            t2 = r20f
            nc.vector.tensor_mul(t2[:], r20a[:], r20a[:])
            nc.vector.scalar_tensor_tensor(t1[:], t1[:], 2.0 / B, t2[:],
                                           ALU.mult, ALU.add)
            nc.vector.scalar_tensor_tensor(t1[:], r20c[:], 1.0 / B, t1[:],
                                           ALU.mult, ALU.add)
            nc.vector.tensor_mul(t2[:], mu2[:], mu2[:])
            var2 = r20c
            nc.vector.tensor_sub(var2[:], t1[:], t2[:])
            lv2 = r20e
            nc.scalar.activation(lv2[:], var2[:], AF.Ln, bias=k_er[:], scale=1.0)
            rs2 = r20f
            nc.scalar.activation(rs2[:], lv2[:], AF.Exp, bias=0.0, scale=-0.5)
            a2r = r20b
            nc.vector.tensor_mul(a2r[:], rs2[:], k_g2r)
            t3 = r20e
            nc.vector.tensor_sub(t3[:], r20a[:], mu2[:])
            nc.vector.tensor_mul(t3[:], t3[:], a2r[:])
            b2r_ = r20f
            nc.vector.tensor_add(b2r_[:], t3[:], k_b2r)
            nc.tensor.transpose(abT[:, 0:1], a2r[:], k_id32[0:1, 0:1])
            nc.tensor.transpose(abT[:, 1:2], b2r_[:], k_id32[0:1, 0:1])
            nc.vector.tensor_copy(absb[:], abT)
            nc.vector.memset(ab128[:], 0.0)
            for rr in range(2):
                nc.sync.dma_start(out=ab128[64 * rr:64 * rr + 20, :], in_=absb[:])
        # --- 3a: z = gelu(a2*zpre+bias), z^2; 4 chunks per ACT op ---
        with ExitStack() as ctx_a:
            ps_z = ctx_a.enter_context(tc.tile_pool(name="psz", bufs=2, space="PSUM"))
            for q in range(NCH // 4):
                zp4 = ps_z.tile([128, 2 * FC], F32, tag="zp4")
                for r in range(4):
                    c = 4 * q + r
                    nc.tensor.matmul(zp4[64 * (c % 2):64 * (c % 2) + 20,
                                         (r // 2) * FC:(r // 2 + 1) * FC],
                                     W3h[:], projT[:, c * FC:(c + 1) * FC],
                                     start=True, stop=True)
                gs = q * 2 * FC
                nc.scalar.activation(z_all[:, gs:gs + 2 * FC], zp4[:], AF.Gelu,
                                     bias=ab128[:, 1:2], scale=ab128[:, 0:1])
                for r in range(2):
                    zr = 64 * r
                    nc.vector.tensor_mul(z_all[zr + 32:zr + 52, gs:gs + 2 * FC],
                                         z_all[zr:zr + 20, gs:gs + 2 * FC],
                                         z_all[zr:zr + 20, gs:gs + 2 * FC])
        # --- 3b: fuzzy memberships + head, per chunk-pair (1024 batch) ---
        pool = ctx.enter_context(tc.tile_pool(name="p3", bufs=3))
        ps_u1 = ctx.enter_context(tc.tile_pool(name="psu1", bufs=2, space="PSUM"))
        ps_u2 = ctx.enter_context(tc.tile_pool(name="psu2", bufs=1, space="PSUM"))
        ps_o = ctx.enter_context(tc.tile_pool(name="pso", bufs=1, space="PSUM"))
        orow = {}
        utiles = {}

        def emit_umm(i):
            u1 = ps_u1.tile([128, 2 * FC], F32, tag="u1")
            u2 = ps_u2.tile([72, 2 * FC], F32, tag="u2")
            utiles[i] = (u1, u2)
            for h in range(2):
                c = 2 * i + h
                g, r = c // 2, c % 2
                zsl = z_all[64 * r:64 * r + 52, g * FC:(g + 1) * FC]
                hs = h * FC
                nc.tensor.matmul(u1[:, hs:hs + FC],
                                 k_azs1[64 * r:64 * r + 52, :],
                                 zsl, start=True, stop=True)
                nc.tensor.matmul(u2[:, hs:hs + FC],
                                 k_azs2[64 * r:64 * r + 52, :],
                                 zsl, start=True, stop=True)

        def emit_tail(i):
            u1, u2 = utiles.pop(i)
            e1u = pool.tile([128, 2 * FC], F16, tag="e1u")
            nc.scalar.activation(e1u[:], u1[:], AF.Exp, bias=k_ub1, scale=-0.5)
            e1l = pool.tile([128, 2 * FC], F16, tag="e1l")
            nc.scalar.activation(e1l[:], u1[:], AF.Exp, bias=k_lb1, scale=k_ls1)
            e2u = pool.tile([72, 2 * FC], F16, tag="e2u")
            nc.scalar.activation(e2u[:], u2[:], AF.Exp, bias=k_ub2, scale=-0.5)
            e2l = pool.tile([72, 2 * FC], F16, tag="e2l")
            nc.scalar.activation(e2l[:], u2[:], AF.Exp, bias=k_lb2, scale=k_ls2)
            if i % 2 == 0:
                orow_t = ps_o.tile([128, 2 * FC], F32, tag="orow")
                orow[0] = orow_t
            rr = 64 * (i % 2)
            for h in range(2):
                hs = h * FC
                nc.tensor.matmul(orow[0][rr:rr + 1, hs:hs + FC], k_wh1,
                                 e1u[:, hs:hs + FC], start=True, stop=False)
                nc.tensor.matmul(orow[0][rr:rr + 1, hs:hs + FC], k_wh1,
                                 e1l[:, hs:hs + FC], start=False, stop=False)
                nc.tensor.matmul(orow[0][rr:rr + 1, hs:hs + FC], k_wh2,
                                 e2u[:, hs:hs + FC], start=False, stop=False)
                nc.tensor.matmul(orow[0][rr:rr + 1, hs:hs + FC], k_wh2,
                                 e2l[:, hs:hs + FC], start=False, stop=True)
            if i % 2 == 1:
                g2 = i // 2
                outsb = pool.tile([128, 2 * FC], F32, tag="outsb")
                nc.vector.tensor_scalar_add(outsb[:], orow[0][:], k_hb[:])
                nc.sync.dma_start(
                    out=out[:, :].rearrange("(g r q) one -> g r (q one)",
                                            r=2, q=2 * FC)[g2],
                    in_=outsb[:].rearrange("(r k) q -> r k q", k=64)[:, 0])

        for i in range(NPAIR + 1):
            if i < NPAIR:
                emit_umm(i)
            if i > 0:
                emit_tail(i - 1)
    octx.close()
    _split_multiwaits(nc)
    return nc


def _split_multiwaits(nc, max_waits=1):
    # hoist extra sync waits into single-wait nops placed just before the
    # offending instruction (walrus here rejects multi-wait instructions)
    for bb in nc.m.functions[0].blocks:
        insts = bb.instructions
        i = 0
        while i < len(insts):
            inst = insts[i]
            si = getattr(inst, "sync_info", None)
            waits = list(si.on_wait) if si and si.on_wait else []
            if len(waits) > max_waits:
                inst.sync_info = mybir.SyncInfo(
                    on_wait=waits[:max_waits], on_update=si.on_update)
                for j, w in enumerate(waits[max_waits:]):
                    n = mybir.InstNoOp(name=f"{inst.name}_ws{j}", ins=[], outs=[])
                    n.engine = inst.engine
                    n.sync_info = mybir.SyncInfo(on_wait=[w], on_update=[])
                    nc.register_instruction(n, overwrite=True)
                    insts.insert(i, n)
                    i += 1
            i += 1


LAST = None


def kernel(**inputs):
    global LAST
    import os
    x = np.asarray(inputs["x"], np.float32)
    p = {k: np.asarray(v) for k, v in inputs.items() if k != "x"}
    nc = _build(p)
    in_maps = [{"x": np.ascontiguousarray(x[i * BS:(i + 1) * BS])}
               for i in range(NCORES)]
    kw = {}
    tdir = os.environ.get("KANFIS_TRACE")
    if tdir:
        os.makedirs(tdir, exist_ok=True)
        kw = dict(trace=True, tmpdir=tdir)
    res = run_bass_kernel_spmd(nc, in_maps, core_ids=list(range(NCORES)), **kw)
    LAST = res
    return np.concatenate([res.results[i]["out"] for i in range(NCORES)], axis=0)


# revision 5
# speedup vs baseline: 1.0438x; 1.0247x over previous
"""KANFIS forward on 8 NeuronCores, data-parallel over the batch — v2.

Key differences vs v1:
  * RBF gaussians via exp of a LINEAR form: e = exp(c_k/s^2 * x - 0.5/s^2 * x^2
    + bias), with [x; x^2] stacked on 128 partitions and the quadratic built by
    a single PE matmul (f32r) per k-pair. No per-k ACT Square pass.
  * k-values pair-packed: 4 ACT exps of [128,512] per chunk instead of 8+8
    ops of [64,512].
  * fp16 / f32r matmuls: 1 cycle/row on PE instead of 4 (fp32).
  * Single AllReduce: BN1 stats (S1,Q1 via bn_stats/bn_aggr) plus the
    second-moment matrix M = proj^T proj ride one [128,130] collective;
    BN2 statistics are derived from (S1, M) on-device because layer 2 is
    linear in proj.
  * BN affines folded into matmul weights / activation scale+bias; proj_b and
    fp_b dropped entirely (they cancel inside BatchNorm).
  * z kept feature-major [20-per-chunk rows], gelu packed 4 chunks per ACT op;
    fuzzy memberships and head reduction per 1024-batch pair.
  * Element-wise work spread across ACT / DVE / GPSIMD.
"""
import numpy as np
from contextlib import ExitStack

import concourse.bass as bass
import concourse.tile as tile
from concourse import mybir
from concourse.vector_clock import ScopedClock
from concourse.bass_utils import run_bass_kernel_spmd

F32 = mybir.dt.float32
F32R = mybir.dt.float32r
F16 = mybir.dt.float16
AF = mybir.ActivationFunctionType
ALU = mybir.AluOpType

NCORES = 8
B = 131072
BS = B // NCORES          # 16384 rows per core
G, GS, K, O = 8, 8, 8, 16
TOT, R, FIN = 128, 10, 20
EPS = 1e-5
FC = 512                  # chunk free size
NCH = BS // FC            # 32 chunks
NPAIR = NCH // 2          # 16 chunk-pairs in phase 3b
NG = NCH // 2             # 16 gelu groups of 2 chunks


class SplitDrainTileContext(tile.TileContext):
    """walrus on this stack rejects >1 sync wait per instruction; split the
    kernel-tail drain's waits into single-wait nops."""

    def _drain_and_barrier(self, tick_clock, wait_clock):
        nc = self.nc
        nop = nc.sync.nop(nofuse=True)
        wait_clock.add_sem_waits(nop.ins, ScopedClock({None: tick_clock.global_clock}))
        si = nop.ins.sync_info
        waits = list(si.on_wait) if si and si.on_wait else []
        if len(waits) > 1:
            nop.ins.sync_info = mybir.SyncInfo(on_wait=waits[:1], on_update=si.on_update)
            for w in waits[1:]:
                n2 = nc.sync.nop(nofuse=True)
                n2.ins.sync_info = mybir.SyncInfo(on_wait=[w], on_update=[])
        nc.sync.drain()
        nc.all_engine_barrier()
        assert self.sems is not None
        popped = nc._tile_sem_poison_stack.pop()
        assert popped is self._sem_poison
        nc.clear_and_free_semaphores(list(self.sems.allocated().values()))
        nc.all_engine_barrier()


def _build(p):
    nc = bass.Bass(num_devices=NCORES)
    x = nc.dram_tensor("x", [BS, 64], F32, kind="ExternalInput")
    out = nc.dram_tensor("out", [BS, 1], F32, kind="ExternalOutput")
    ar_in = nc.dram_tensor("ar_in", [128, 130], F16)
    ar_out = nc.dram_tensor("ar_out", [1024, 130], F16)

    # ---- baked constants (numpy) ----
    sig = np.exp(p["rbf_log_widths"]) + 1e-6            # [G,K]
    inv2 = (1.0 / sig ** 2).astype(np.float64)
    cen = p["rbf_centres"].astype(np.float64)
    pw = p["proj_W"]                                    # [G,O,GS]
    rw = p["rbf_weights"]                               # [G,K]
    # u-matmul lhsT [128, 4*128] f32 (used as f32r): rows 0-63 x_f, 64-127 x2_f
    # col (pair p, m = kk*64+f): u = (c/s^2) x - (0.5/s^2) x^2
    ku = np.zeros((128, 4 * 128), np.float32)
    keb = np.zeros((128, 4), np.float32)                # exp bias per pair
    for pp_ in range(4):
        for kk in range(2):
            k = 2 * pp_ + kk
            for f in range(64):
                g = f // GS
                m = kk * 64 + f
                ku[f, pp_ * 128 + m] = cen[g, k] * inv2[g, k]
                ku[64 + f, pp_ * 128 + m] = -0.5 * inv2[g, k]
                keb[m, pp_] = -0.5 * cen[g, k] ** 2 * inv2[g, k]
    # proj lhsT per pair [128, 4*128] f16: rows m=(kk,f) -> cols go.
    # The exp bias exp(-0.5 c^2/s^2) is folded multiplicatively into the rows
    # so all four per-pair exps share scale=1, bias=0 and merge into one op.
    klh = np.zeros((128, 4 * 128), np.float32)
    for pp_ in range(4):
        for kk in range(2):
            k = 2 * pp_ + kk
            for g in range(G):
                eb = np.exp(-0.5 * cen[g, k] ** 2 * inv2[g, k])
                klh[kk * 64 + g * GS:(kk * 64 + (g + 1) * GS),
                    pp_ * 128 + g * O:pp_ * 128 + (g + 1) * O] = \
                    pw[g].T * rw[g, k] * eb
    # linear term lhsT [128,128] (rhs = X2; x2 rows get zero coef)
    klin = np.zeros((128, 128), np.float32)
    for g in range(G):
        klin[g * GS:(g + 1) * GS, g * O:(g + 1) * O] = pw[g].T * p["rbf_linear_w"][g]

    su = np.exp(p["fz_log_su"]) + 1e-6                  # [R,FIN]
    sl = np.minimum(np.exp(p["fz_log_sl"]) + 1e-6, su * 0.9)
    cz = p["fz_centres"]
    # fuzzy u lhsT: z-part and z2-part, [20, 200] each -> split 128/72, tiled x4
    afz_z = np.zeros((FIN, 200), np.float32)
    afz_z2 = np.zeros((FIN, 200), np.float32)
    for r in range(R):
        for f in range(FIN):
            m = r * FIN + f
            afz_z[f, m] = -2.0 * cz[r, f] / su[r, f] ** 2
            afz_z2[f, m] = 1.0 / su[r, f] ** 2
    ubias = (-0.5 * cz ** 2 / su ** 2).reshape(200, 1).astype(np.float32)
    lbias = (-0.5 * cz ** 2 / sl ** 2).reshape(200, 1).astype(np.float32)
    lscale = (-0.5 * (su / sl) ** 2).reshape(200, 1).astype(np.float32)
    wh = np.repeat(p["head_W"].reshape(R, 1) * 0.5 / FIN, FIN, 0).astype(np.float32)
    head_b = float(np.asarray(p["head_b"]).reshape(-1)[0])

    def it(name, arr, dt=np.float32):
        return nc.inline_tensor(np.ascontiguousarray(arr, dt), name=name)

    def pad128(a):
        o = np.zeros((128, a.shape[1]), a.dtype)
        o[:a.shape[0]] = a
        return o

    f32_parts = [
        ("keb", keb), ("id32", np.eye(128, dtype=np.float32)),
        ("ones", np.ones((128, 1), np.float32)),
        ("g1", p["bn1_gamma"].reshape(128, 1)),
        ("b1", p["bn1_beta"].reshape(128, 1)),
        ("fpw", p["fp_W"].T),
        ("ub1", ubias[:128]), ("lb1", lbias[:128]), ("ls1", lscale[:128]),
        ("ub2", pad128(ubias[128:])), ("lb2", pad128(lbias[128:])),
        ("ls2", pad128(lscale[128:])),
        ("g2r", pad128(p["bn2_gamma"].reshape(1, 20))),
        ("b2r", pad128(p["bn2_beta"].reshape(1, 20))),
    ]
    def blk2s(az, az2):
        # rows 0-19: z coefs, 32-51: z^2 coefs; replicated at +64
        o = np.zeros((128, az.shape[1]), np.float32)
        for r_ in range(2):
            o[64 * r_:64 * r_ + FIN] = az
            o[64 * r_ + 32:64 * r_ + 32 + FIN] = az2
        return o

    f16_parts = [
        ("ku", ku.astype(np.float16)),
        ("klh", klh.astype(np.float16)),
        ("klin", klin.astype(np.float16)),
        ("idh", np.eye(128, dtype=np.float16)),
        ("azs1", blk2s(afz_z[:, :128], afz_z2[:, :128]).astype(np.float16)),
        ("azs2", blk2s(afz_z[:, 128:], afz_z2[:, 128:]).astype(np.float16)),
        ("wh1", wh[:128].astype(np.float16)),
        ("wh2", pad128(wh[128:]).astype(np.float16)),
    ]
    f32_off, f16_off = {}, {}
    o = 0
    for nm, a in f32_parts:
        f32_off[nm] = o
        o += a.shape[1]
    nf32 = o
    o = 0
    for nm, a in f16_parts:
        f16_off[nm] = o
        o += a.shape[1]
    nf16 = o
    c_f32 = it("c_f32", np.concatenate([a for _, a in f32_parts], axis=1))
    c_f16 = it("c_f16", np.concatenate([a for _, a in f16_parts], axis=1),
               np.float16)

    octx = ExitStack()

    def sb(n, s, dt=F32):
        return octx.enter_context(nc.sbuf_tensor(n, s, dt))

    projT = sb("projT", [128, BS], F16)        # 4MB persistent
    z_all = sb("z_all", [128, NG * FC], F16)   # z rows 0-19/64-83, z^2 rows 32-51/96-115
    stats = sb("stats", [128, NCH * 6])
    # const sbuf blocks
    kf32 = sb("kf32", [128, nf32])
    kf16 = sb("kf16", [128, nf16], F16)

    def s32(nm, w, rows=128):
        off = f32_off[nm]
        return kf32[0:rows, off:off + w]

    def s16(nm, w, rows=128):
        off = f16_off[nm]
        return kf16[0:rows, off:off + w]

    k_ku = s16("ku", 512); k_keb = s32("keb", 4)
    k_klh = s16("klh", 512); k_klin = s16("klin", 128)
    k_id32 = s32("id32", 128); k_idh = s16("idh", 128)
    k_on = s32("ones", 1)
    k_g1 = s32("g1", 1); k_b1 = s32("b1", 1)
    k_g2r = s32("g2r", 20, 1); k_b2r = s32("b2r", 20, 1)
    k_fpw = s32("fpw", 20)
    k_azs1 = s16("azs1", 128); k_azs2 = s16("azs2", 72)
    k_ub1 = s32("ub1", 1); k_ub2 = s32("ub2", 1, 72)
    k_lb1 = s32("lb1", 1); k_lb2 = s32("lb2", 1, 72)
    k_ls1 = s32("ls1", 1); k_ls2 = s32("ls2", 1, 72)
    k_wh1 = s16("wh1", 1); k_wh2 = s16("wh2", 1, 72)

    def s16_ku(pr):
        off = f16_off["ku"]
        return kf16[:, off + pr * 128:off + (pr + 1) * 128]

    def s16_klh(pr):
        off = f16_off["klh"]
        return kf16[:, off + pr * 128:off + (pr + 1) * 128]

    def s32_keb(pr):
        off = f32_off["keb"]
        return kf32[:, off + pr:off + pr + 1]
    k_hb = sb("k_hb", [128, 1])
    k_e1 = sb("k_e1", [128, 1]); k_er = sb("k_er", [1, 1])
    s1q1 = sb("s1q1", [128, 2], F16)
    mvsb = sb("mvsb", [128, 2])
    msb = sb("msb", [128, 128], F16)
    arsb = sb("arsb", [128, 130])
    agsb = sb("agsb", [128, 1040], F16); scr520 = sb("scr520", [128, 520])
    W3f = sb("W3f", [128, 20]); W3h = sb("W3h", [128, 20], F16)
    a1v = sb("a1v", [128, 1]); d1v = sb("d1v", [128, 1])
    ab2 = sb("ab2", [2, 20]); absb = sb("absb", [20, 2]); ab128 = sb("ab128", [128, 2])
    WTsb = sb("WTsb", [128, 20])
    sm1 = sb("sm1", [128, 1]); sm2 = sb("sm2", [128, 1]); sm3 = sb("sm3", [128, 1])
    r20a = sb("r20a", [1, 20]); r20b = sb("r20b", [1, 20]); r20c = sb("r20c", [1, 20])
    r20d = sb("r20d", [1, 20]); r20e = sb("r20e", [1, 20]); r20f = sb("r20f", [1, 20])

    # ================= phase 1 =================
    with ExitStack() as ctx:
        tc = ctx.enter_context(SplitDrainTileContext(nc))
        nc.sync.dma_start(out=kf32[:], in_=c_f32[:, :])
        nc.sync.dma_start(out=kf16[:], in_=c_f16[:, :])
        nc.vector.memset(k_hb[:], head_b)
        nc.vector.memset(k_e1[:], EPS)
        nc.vector.memset(k_er[:], EPS)
        pool = ctx.enter_context(tc.tile_pool(name="p1", bufs=3))
        poolx = ctx.enter_context(tc.tile_pool(name="p1x", bufs=6))
        ps_x = ctx.enter_context(tc.tile_pool(name="psx", bufs=1, space="PSUM"))
        ps_u = ctx.enter_context(tc.tile_pool(name="psu", bufs=1, space="PSUM"))
        ps_p = ctx.enter_context(tc.tile_pool(name="psp", bufs=1, space="PSUM"))
        ps_b = ctx.enter_context(tc.tile_pool(name="psb", bufs=1, space="PSUM"))
        ps_m = ctx.enter_context(tc.tile_pool(name="psm", bufs=1, space="PSUM"))
        Mps = ps_m.tile([128, 128], F32, tag="M")
        xv = x.rearrange("(c j p) f -> c p j f", j=4, p=128)

        def emit_mwork(c2):
            # covers chunks 2*c2, 2*c2+1 (1024 batch rows)
            ccs = 2 * c2 * FC
            pbp = ps_b.tile([128, 2 * FC], F16, tag="pbp")
            for j in range(8):
                nc.tensor.transpose(pbp[:, j * 128:(j + 1) * 128],
                                    projT[:, ccs + j * 128:ccs + (j + 1) * 128],
                                    k_idh)
            pbs = pool.tile([128, 2 * FC], F16, tag="pbs")
            nc.scalar.activation(pbs[:], pbp[:], AF.Identity, bias=0.0, scale=1.0)
            for j in range(8):
                nc.tensor.matmul(Mps[:], pbs[:, j * 128:(j + 1) * 128],
                                 pbs[:, j * 128:(j + 1) * 128],
                                 start=(c2 == 0 and j == 0),
                                 stop=(c2 == NCH // 2 - 1 and j == 7))

        etiles = {}

        def emit_head(c):
            # x load, transpose, X2 build, u-matmuls, exps for chunk c
            xt4 = poolx.tile([128, 256], F32, tag="xt4")
            nc.sync.dma_start(out=xt4[:].rearrange("p (j f) -> p j f", j=4),
                              in_=xv[c])
            xt4h = pool.tile([128, 256], F16, tag="xt4h")
            nc.gpsimd.tensor_copy(xt4h[:], xt4[:])
            xtp = ps_x.tile([64, FC], F16, tag="xtp")
            for j in range(4):
                nc.tensor.transpose(xtp[:, j * 128:(j + 1) * 128],
                                    xt4h[:, j * 64:(j + 1) * 64], k_idh)
            X2 = pool.tile([128, FC], F16, tag="X2")
            nc.vector.tensor_copy(X2[0:64, :], xtp[:])
            nc.gpsimd.tensor_mul(X2[64:128, :], X2[0:64, :], X2[0:64, :])
            u4a = ps_u.tile([128, 2 * FC], F32, tag="u4a")
            u4b = ps_u.tile([128, 2 * FC], F32, tag="u4b")
            for pr in range(4):
                ut = u4a if pr < 2 else u4b
                nc.tensor.matmul(ut[:, (pr % 2) * FC:(pr % 2 + 1) * FC],
                                 s16_ku(pr), X2[:], start=True, stop=True)
            e4a = pool.tile([128, 2 * FC], F16, tag="e4a")
            nc.scalar.activation(e4a[:], u4a[:], AF.Exp, bias=0.0, scale=1.0)
            e4b = pool.tile([128, 2 * FC], F16, tag="e4b")
            nc.scalar.activation(e4b[:], u4b[:], AF.Exp, bias=0.0, scale=1.0)
            etiles[c] = (X2, e4a, e4b)

        def emit_pp(c):
            cs = c * FC
            X2, e4a, e4b = etiles.pop(c)
            pp = ps_p.tile([128, FC], F32, tag="pp")
            nc.tensor.matmul(pp[:], k_klin, X2[:], start=True, stop=False)
            for pr in range(4):
                et = e4a if pr < 2 else e4b
                nc.tensor.matmul(pp[:], s16_klh(pr),
                                 et[:, (pr % 2) * FC:(pr % 2 + 1) * FC],
                                 start=False, stop=(pr == 3))
            nc.vector.tensor_copy(projT[:, cs:cs + FC], pp[:])
            nc.vector.bn_stats(stats[:, c * 6:(c + 1) * 6], pp[:])

        for c in range(NCH + 1):
            if c < NCH:
                emit_head(c)
            if c > 0:
                emit_pp(c - 1)
            if c > 2 and c % 2 == 1:
                emit_mwork(c // 2 - 1)
        emit_mwork(NCH // 2 - 1)
        # local BN1 stats -> S1, Q1
        nc.vector.bn_aggr(mvsb[:], stats[:].rearrange("p (r s) -> p r s", s=6))
        nc.vector.tensor_mul(sm1[:], mvsb[:, 0:1], mvsb[:, 0:1])      # mean^2
        nc.vector.tensor_add(sm2[:], mvsb[:, 1:2], sm1[:])            # var+mean^2
        nc.vector.tensor_scalar_mul(s1q1[:, 0:1], mvsb[:, 0:1], float(BS) / 64.0)
        nc.vector.tensor_scalar_mul(s1q1[:, 1:2], sm2[:], float(BS) / 64.0)
        nc.vector.tensor_scalar_mul(msb[:], Mps[:], 1.0 / 64.0)
        nc.sync.dma_start(out=ar_in[:, 0:2], in_=s1q1[:])
        nc.sync.dma_start(out=ar_in[:, 2:130], in_=msb[:])

    with nc.semaphore("cc1") as cs_:
        nc.gpsimd.collective_compute(
            "AllGather", ALU.bypass, replica_groups=[list(range(NCORES))],
            ins=[ar_in[:, :].opt()], outs=[ar_out[:, :].opt()]).then_inc(cs_, 1)
        nc.gpsimd.wait_ge(cs_, 1)
        nc.all_engine_barrier()

    # ================= phase 3 =================
    with ExitStack() as ctx:
        tc = ctx.enter_context(SplitDrainTileContext(nc))
        # --- phase 2: derive BN1 scale/shift and BN2 stats from (S1,Q1,M) ---
        with ExitStack() as ctx2:
            pool2 = ctx2.enter_context(tc.tile_pool(name="p2", bufs=1))
            ps2 = ctx2.enter_context(tc.tile_pool(name="ps2", bufs=1, space="PSUM"))
            nc.sync.dma_start(out=agsb[:].rearrange("p (i c) -> p i c", i=8),
                              in_=ar_out.rearrange("(i p) c -> p i c", i=8))
            nc.vector.tensor_add(scr520[:], agsb[:, 0:520], agsb[:, 520:1040])
            nc.vector.tensor_add(scr520[:, 0:260], scr520[:, 0:260],
                                 scr520[:, 260:520])
            nc.vector.tensor_add(arsb[:], scr520[:, 0:130], scr520[:, 130:260])
            nc.vector.tensor_scalar_mul(arsb[:], arsb[:], 64.0)
            mu = pool2.tile([128, 1], F32, tag="mu")
            nc.vector.tensor_scalar_mul(mu[:], arsb[:, 0:1], 1.0 / B)
            nc.vector.tensor_mul(sm1[:], mu[:], mu[:])
            var1 = pool2.tile([128, 1], F32, tag="var1")
            nc.vector.scalar_tensor_tensor(var1[:], arsb[:, 1:2], 1.0 / B, sm1[:],
                                           ALU.mult, ALU.subtract)
            lv = pool2.tile([128, 1], F32, tag="lv")
            nc.scalar.activation(lv[:], var1[:], AF.Ln, bias=k_e1[:], scale=1.0)
            rs1 = pool2.tile([128, 1], F32, tag="rs1")
            nc.scalar.activation(rs1[:], lv[:], AF.Exp, bias=0.0, scale=-0.5)
            nc.vector.tensor_mul(a1v[:], rs1[:], k_g1)
            nc.vector.tensor_mul(sm3[:], mu[:], a1v[:])
            nc.vector.scalar_tensor_tensor(d1v[:], sm3[:], -1.0, k_b1,
                                           ALU.mult, ALU.add)
            nc.scalar.activation(W3f[:], k_fpw, AF.Identity, bias=0.0, scale=a1v[:])
            nc.vector.tensor_copy(W3h[:], W3f[:])
            p2ps = ps2.tile([128, 128], F32, tag="p2ps")
            T1 = p2ps[:, 0:20]
            qw = p2ps[0:1, 20:40]
            cy = p2ps[0:1, 40:60]
            sw = p2ps[0:1, 60:80]
            abT = p2ps[0:20, 80:82]
            nc.tensor.matmul(T1, arsb[:, 2:130], W3f[:], start=True, stop=True)
            nc.vector.tensor_mul(WTsb[:], W3f[:], T1)
            nc.tensor.matmul(qw, k_on, WTsb[:], start=True, stop=True)
            nc.tensor.matmul(cy, d1v[:], k_fpw, start=True, stop=True)
            nc.tensor.matmul(sw, arsb[:, 0:1], W3f[:], start=True, stop=True)
            nc.vector.tensor_copy(r20a[:], cy)       # cy
            nc.vector.tensor_copy(r20b[:], sw)       # S1@W3
            nc.vector.tensor_copy(r20c[:], qw)       # diag(W3^T M W3)
            mu2 = r20d
            nc.vector.scalar_tensor_tensor(mu2[:], r20b[:], 1.0 / B, r20a[:],
                                           ALU.mult, ALU.add)
            t1 = r20e
            nc.vector.tensor_mul(t1[:], r20a[:], r20b[:])
            t2 = r20f
            nc.vector.tensor_mul(t2[:], r20a[:], r20a[:])
            nc.vector.scalar_tensor_tensor(t1[:], t1[:], 2.0 / B, t2[:],
                                           ALU.mult, ALU.add)
            nc.vector.scalar_tensor_tensor(t1[:], r20c[:], 1.0 / B, t1[:],
                                           ALU.mult, ALU.add)
            nc.vector.tensor_mul(t2[:], mu2[:], mu2[:])
            var2 = r20c
            nc.vector.tensor_sub(var2[:], t1[:], t2[:])
            lv2 = r20e
            nc.scalar.activation(lv2[:], var2[:], AF.Ln, bias=k_er[:], scale=1.0)
            rs2 = r20f
            nc.scalar.activation(rs2[:], lv2[:], AF.Exp, bias=0.0, scale=-0.5)
            a2r = r20b
            nc.vector.tensor_mul(a2r[:], rs2[:], k_g2r)
            t3 = r20e
            nc.vector.tensor_sub(t3[:], r20a[:], mu2[:])
            nc.vector.tensor_mul(t3[:], t3[:], a2r[:])
            b2r_ = r20f
            nc.vector.tensor_add(b2r_[:], t3[:], k_b2r)
            nc.tensor.transpose(abT[:, 0:1], a2r[:], k_id32[0:1, 0:1])
            nc.tensor.transpose(abT[:, 1:2], b2r_[:], k_id32[0:1, 0:1])
            nc.vector.tensor_copy(absb[:], abT)
            nc.vector.memset(ab128[:], 0.0)
            for rr in range(2):
                nc.sync.dma_start(out=ab128[64 * rr:64 * rr + 20, :], in_=absb[:])
        # --- 3a: z = gelu(a2*zpre+bias), z^2; 4 chunks per ACT op ---
        with ExitStack() as ctx_a:
            ps_z = ctx_a.enter_context(tc.tile_pool(name="psz", bufs=2, space="PSUM"))
            for q in range(NCH // 4):
                zp4 = ps_z.tile([128, 2 * FC], F32, tag="zp4")
                for r in range(4):
                    c = 4 * q + r
                    nc.tensor.matmul(zp4[64 * (c % 2):64 * (c % 2) + 20,
                                         (r // 2) * FC:(r // 2 + 1) * FC],
                                     W3h[:], projT[:, c * FC:(c + 1) * FC],
                                     start=True, stop=True)
                gs = q * 2 * FC
                nc.scalar.activation(z_all[:, gs:gs + 2 * FC], zp4[:], AF.Gelu,
                                     bias=ab128[:, 1:2], scale=ab128[:, 0:1])
                for r in range(2):
                    zr = 64 * r
                    nc.vector.tensor_mul(z_all[zr + 32:zr + 52, gs:gs + 2 * FC],
                                         z_all[zr:zr + 20, gs:gs + 2 * FC],
                                         z_all[zr:zr + 20, gs:gs + 2 * FC])
        # --- 3b: fuzzy memberships + head, per chunk-pair (1024 batch) ---
        pool = ctx.enter_context(tc.tile_pool(name="p3", bufs=3))
        ps_u1 = ctx.enter_context(tc.tile_pool(name="psu1", bufs=2, space="PSUM"))
        ps_u2 = ctx.enter_context(tc.tile_pool(name="psu2", bufs=1, space="PSUM"))
        ps_o = ctx.enter_context(tc.tile_pool(name="pso", bufs=1, space="PSUM"))
        orow = {}
        utiles = {}

        def emit_umm(i):
            u1 = ps_u1.tile([128, 2 * FC], F32, tag="u1")
            u2 = ps_u2.tile([72, 2 * FC], F32, tag="u2")
            utiles[i] = (u1, u2)
            for h in range(2):
                c = 2 * i + h
                g, r = c // 2, c % 2
                zsl = z_all[64 * r:64 * r + 52, g * FC:(g + 1) * FC]
                hs = h * FC
                nc.tensor.matmul(u1[:, hs:hs + FC],
                                 k_azs1[64 * r:64 * r + 52, :],
                                 zsl, start=True, stop=True)
                nc.tensor.matmul(u2[:, hs:hs + FC],
                                 k_azs2[64 * r:64 * r + 52, :],
                                 zsl, start=True, stop=True)

        def emit_tail(i):
            u1, u2 = utiles.pop(i)
            e1u = pool.tile([128, 2 * FC], F16, tag="e1u")
            nc.scalar.activation(e1u[:], u1[:], AF.Exp, bias=k_ub1, scale=-0.5)
            e1l = pool.tile([128, 2 * FC], F16, tag="e1l")
            nc.scalar.activation(e1l[:], u1[:], AF.Exp, bias=k_lb1, scale=k_ls1)
            e2u = pool.tile([72, 2 * FC], F16, tag="e2u")
            nc.scalar.activation(e2u[:], u2[:], AF.Exp, bias=k_ub2, scale=-0.5)
            e2l = pool.tile([72, 2 * FC], F16, tag="e2l")
            nc.scalar.activation(e2l[:], u2[:], AF.Exp, bias=k_lb2, scale=k_ls2)
            if i % 2 == 0:
                orow_t = ps_o.tile([128, 2 * FC], F32, tag="orow")
                orow[0] = orow_t
            rr = 64 * (i % 2)
            for h in range(2):
                hs = h * FC
                nc.tensor.matmul(orow[0][rr:rr + 1, hs:hs + FC], k_wh1,
                                 e1u[:, hs:hs + FC], start=True, stop=False)
                nc.tensor.matmul(orow[0][rr:rr + 1, hs:hs + FC], k_wh1,
                                 e1l[:, hs:hs + FC], start=False, stop=False)
                nc.tensor.matmul(orow[0][rr:rr + 1, hs:hs + FC], k_wh2,
                                 e2u[:, hs:hs + FC], start=False, stop=False)
                nc.tensor.matmul(orow[0][rr:rr + 1, hs:hs + FC], k_wh2,
                                 e2l[:, hs:hs + FC], start=False, stop=True)
            if i % 2 == 1:
                g2 = i // 2
                outsb = pool.tile([128, 2 * FC], F32, tag="outsb")
                nc.vector.tensor_scalar_add(outsb[:], orow[0][:], k_hb[:])
                nc.sync.dma_start(
                    out=out[:, :].rearrange("(g r q) one -> g r (q one)",
                                            r=2, q=2 * FC)[g2],
                    in_=outsb[:].rearrange("(r k) q -> r k q", k=64)[:, 0])

        for i in range(NPAIR + 1):
            if i < NPAIR:
                emit_umm(i)
            if i > 0:
                emit_tail(i - 1)
    octx.close()
    _split_multiwaits(nc)
    return nc


def _split_multiwaits(nc, max_waits=1):
    # hoist extra sync waits into single-wait nops placed just before the
    # offending instruction (walrus here rejects multi-wait instructions)
    for bb in nc.m.functions[0].blocks:
        insts = bb.instructions
        i = 0
        while i < len(insts):
            inst = insts[i]
            si = getattr(inst, "sync_info", None)
            waits = list(si.on_wait) if si and si.on_wait else []
            if len(waits) > max_waits:
                inst.sync_info = mybir.SyncInfo(
                    on_wait=waits[:max_waits], on_update=si.on_update)
                for j, w in enumerate(waits[max_waits:]):
                    n = mybir.InstNoOp(name=f"{inst.name}_ws{j}", ins=[], outs=[])
                    n.engine = inst.engine
                    n.sync_info = mybir.SyncInfo(on_wait=[w], on_update=[])
                    nc.register_instruction(n, overwrite=True)
                    insts.insert(i, n)
                    i += 1
            i += 1


LAST = None


def kernel(**inputs):
    global LAST
    import os
    x = np.asarray(inputs["x"], np.float32)
    p = {k: np.asarray(v) for k, v in inputs.items() if k != "x"}
    nc = _build(p)
    in_maps = [{"x": np.ascontiguousarray(x[i * BS:(i + 1) * BS])}
               for i in range(NCORES)]
    kw = {}
    tdir = os.environ.get("KANFIS_TRACE")
    if tdir:
        os.makedirs(tdir, exist_ok=True)
        kw = dict(trace=True, tmpdir=tdir)
    res = run_bass_kernel_spmd(nc, in_maps, core_ids=list(range(NCORES)), **kw)
    LAST = res
    return np.concatenate([res.results[i]["out"] for i in range(NCORES)], axis=0)
